# revision 11
# baseline (speedup 1.0000x reference)
"""Trainium2 Bass kernel for nn_MessageTemporalEncoding.

Math (per edge e, head h, pair k, with d = 64*h + 2*k + parity):
  tn    = a*t + b                      (a = t_scale/(sqrt(T_VAR)+1e-6), b = t_shift)
  ang   = tn * w[h,k]                  (w = 1/exp(rope_log_ts))
  c,s   = cos(ang), sin(ang)
  g     = sigmoid(-lam_h*|tn| + bias_h)
  out   = g*rot(msg) + (1-g)*msg + feat @ W + fb
Using rot linearity (gate is constant within a pair): g*rot(m)+(1-g)*m = m + (R-I)(g*m),
so with Mg = g*m:
  out[2k]   = f[2k]   + m[2k]   + (c-1)*Mg[2k]   - s*Mg[2k+1]
  out[2k+1] = f[2k+1] + m[2k+1] + (c-1)*Mg[2k+1] + s*Mg[2k]

HW Sin only accepts args in [-pi, pi]; we use reflections (valid since |tn| < 4.71):
  cos(x)  = Sin(pi/2 - |x|)                              (|x| <= 3pi/2)
  sin(x)  = Sin(pi*sign(x) - x),  -sin(x) = Sin(x - pi*sign(x))   (|x| <= 2pi)
Fourier phases reach +-55, so the host pre-reduces them: Y = mod(phi + pi, 2pi),
device computes featT = Sin(Y - pi) = sin(phi).

Engine mapping per 128-edge chunk (edges on partitions, DIM on free):
  ACT : cosT [128,256] = Sin(Wrow * (-|tn|) + pi/2)
        sinPM[:,0::2]  = Sin(Wrow * tn - pi*sign(tn))   (= -sin(ang))
        sinPM[:,1::2]  = Sin(Wrow * (-tn) + pi*sign(tn)) (= +sin(ang))
        featT [33,512] = Sin(Y - pi) per 4-chunk group
  DVE : Mg = M * g2 (gate broadcast via step-0 AP)
        u1 = (cosT_dup - 1) * Mg   (fused scalar_tensor_tensor)
  Pool: u2 = swap(Mg) * sinPM      (pair-swap access pattern)
  PE  : psum = featT.T@W33 + I@M + I@u1 + I@u2  (W33 rows: fourier_W + fourier_b)
  DVE/ACT: split-copy psum -> sbuf, DMA out.

Sharding: data-parallel over E across 8 cores; params replicated.
"""

import math
from contextlib import ExitStack

import numpy as np

import concourse.bass as bass
import concourse.bacc as bacc
import concourse.tile as tile
from concourse import mybir

F32 = mybir.dt.float32
AF = mybir.ActivationFunctionType
OP = mybir.AluOpType

E_FULL = 200000
DIM = 512
H = 8
NHK = 256          # pairs total
NF = 16
KF = 2 * NF + 1    # 33 fourier rows (incl. constant row for fourier_b)
N_CORES = 8
P = 128
E_CORE = E_FULL // N_CORES          # 25000
NT = (E_CORE + P - 1) // P          # 196 chunks of 128 edges
E_PAD = NT * P                      # 25088
GROUP = 4                           # chunks per featT group (4*128=512)
ESC = 256                           # escape split: cols [0:ESC] DVE, rest ACT
HALF_PI = math.pi / 2
PI = math.pi
TWO_PI = 2.0 * math.pi
CLAMP = 4.70                        # |tn| beyond this: host recomputes the row


def _swap_pairs(ap3):
    """[128, 256, 2] view with the pair axis reversed: (even,odd)->(odd,even)."""
    try:
        return ap3[:, :, ::-1]
    except Exception:
        sw = ap3.copy()
        ap = [list(d) for d in sw.ap]
        step = ap[-1][0]
        ap[-1][0] = -step
        return bass.AP(tensor=sw.tensor, offset=sw.offset + step, ap=ap)


def build_nc(nt=NT):
    e_pad = nt * P
    ngroups = nt // GROUP
    nc = bacc.Bacc("TRN2", target_bir_lowering=False, debug=False)

    def din(name, shape):
        return nc.dram_tensor(name, shape, F32, kind="ExternalInput").ap()

    msg = din("msg", [e_pad, DIM])
    tn_cm = din("tn_cm", [P, nt])          # +tn  (column-major [p,c] = tn[c*128+p])
    tnneg_cm = din("tnneg_cm", [P, nt])    # -tn
    tnabsneg_cm = din("tnabsneg_cm", [P, nt])  # -|tn|
    pisign_cm = din("pisign_cm", [P, nt])  # +pi*sign(tn)
    pisignneg_cm = din("pisignneg_cm", [P, nt])  # -pi*sign(tn)
    yrows = din("yrows", [KF, e_pad])      # mod(tn*fc + fb + pi, 2pi)
    wrow = din("wrow", [P, NHK])           # w = 1/exp(rope_log_ts), hk order
    w33 = din("w33", [KF, DIM])            # [fourier_W; fourier_b]
    ident = din("ident", [P, P])
    pl2 = din("pl2", [P, H])               # +lam/2 (bcast rows)
    bh2 = din("bh2", [P, H])               # decay_bias/2 (bcast rows)
    out = nc.dram_tensor("out", [e_pad, DIM], F32, kind="ExternalOutput").ap()

    with tile.TileContext(nc) as tc, ExitStack() as ctx:
        singles = ctx.enter_context(tc.tile_pool(name="singles", bufs=1))
        mpool = ctx.enter_context(tc.tile_pool(name="mpool", bufs=4))
        trig = ctx.enter_context(tc.tile_pool(name="trig", bufs=3))
        work = ctx.enter_context(tc.tile_pool(name="work", bufs=3))
        opool = ctx.enter_context(tc.tile_pool(name="opool", bufs=3))
        fpool = ctx.enter_context(tc.tile_pool(name="fpool", bufs=2))
        psum = ctx.enter_context(tc.tile_pool(name="psum", bufs=3, space="PSUM"))

        def load(ap_dram, shape, tag):
            t = singles.tile(shape, F32, tag=tag)
            nc.sync.dma_start(out=t, in_=ap_dram)
            return t

        s_tn = load(tn_cm, [P, nt], "c_tn")
        s_tnn = load(tnneg_cm, [P, nt], "c_tnn")
        s_tan = load(tnabsneg_cm, [P, nt], "c_tan")
        s_ps = load(pisign_cm, [P, nt], "c_ps")
        s_psn = load(pisignneg_cm, [P, nt], "c_psn")
        s_wrow = load(wrow, [P, NHK], "c_wrow")
        s_w33 = load(w33, [KF, DIM], "c_w33")
        s_ident = load(ident, [P, P], "c_ident")
        s_pl2 = load(pl2, [P, H], "c_pl2")
        s_bh2 = load(bh2, [P, H], "c_bh2")

        s_hpi = singles.tile([P, 1], F32, tag="c_hpi")
        nc.vector.memset(s_hpi, HALF_PI)
        s_npi = singles.tile([KF, 1], F32, tag="c_npi")
        nc.vector.memset(s_npi, -PI)

        # one-time gate: g = 0.5*tanh(|tn|*(lam/2)*(-1)... ) rewritten:
        # z = (-|tn|)*(lam/2) + b/2 ; g = 0.5*tanh(z)+0.5
        gate = singles.tile([P, nt, H], F32, tag="c_gate")
        nc.vector.tensor_tensor(
            gate,
            s_tan.unsqueeze(2).broadcast_to((P, nt, H)),
            s_pl2.unsqueeze(1).broadcast_to((P, nt, H)),
            OP.mult,
        )
        nc.vector.tensor_tensor(
            gate, gate, s_bh2.unsqueeze(1).broadcast_to((P, nt, H)), OP.add
        )
        nc.scalar.activation(gate, gate, AF.Tanh)
        nc.vector.tensor_scalar(gate, gate, 0.5, 0.5, OP.mult, OP.add)

        for g_i in range(ngroups):
            y_t = fpool.tile([KF, GROUP * P], F32)
            nc.sync.dma_start(
                out=y_t, in_=yrows[:, g_i * GROUP * P:(g_i + 1) * GROUP * P]
            )
            featT = fpool.tile([KF, GROUP * P], F32)
            nc.scalar.activation(featT, y_t, AF.Sin, bias=s_npi)

            for i in range(GROUP):
                c = g_i * GROUP + i
                m_t = mpool.tile([P, DIM], F32)
                nc.sync.dma_start(out=m_t, in_=msg[c * P:(c + 1) * P, :])

                cosT = trig.tile([P, NHK], F32)
                nc.scalar.activation(
                    cosT, s_wrow, AF.Sin, bias=s_hpi, scale=s_tan[:, c:c + 1])
                sinPM = trig.tile([P, DIM], F32)
                spm3 = sinPM.rearrange("p (a b) -> p a b", b=2)
                nc.scalar.activation(
                    spm3[:, :, 0], s_wrow, AF.Sin,
                    bias=s_psn[:, c:c + 1], scale=s_tn[:, c:c + 1])
                nc.scalar.activation(
                    spm3[:, :, 1], s_wrow, AF.Sin,
                    bias=s_ps[:, c:c + 1], scale=s_tnn[:, c:c + 1])

                # Mg = m * gate (head-block broadcast)
                mg = work.tile([P, DIM], F32)
                nc.vector.tensor_tensor(
                    mg.rearrange("p (h j) -> p h j", h=H),
                    m_t.rearrange("p (h j) -> p h j", h=H),
                    gate[:, c, :].unsqueeze(2).broadcast_to((P, H, DIM // H)),
                    OP.mult,
                )
                mg3 = mg.rearrange("p (a b) -> p a b", b=2)

                # u1 = (cos - 1) * Mg
                u1 = work.tile([P, DIM], F32)
                nc.vector.scalar_tensor_tensor(
                    u1.rearrange("p (a b) -> p a b", b=2),
                    cosT.unsqueeze(2).broadcast_to((P, NHK, 2)),
                    1.0,
                    mg3,
                    OP.subtract,
                    OP.mult,
                )

                # u2 = swap(Mg) * sinPM   (Pool engine)
                u2 = work.tile([P, DIM], F32)
                nc.gpsimd.tensor_tensor(
                    u2.rearrange("p (a b) -> p a b", b=2),
                    _swap_pairs(mg3),
                    spm3,
                    OP.mult,
                )

                # psum = fourier + M + u1 + u2
                pf = psum.tile([P, DIM], F32)
                nc.tensor.matmul(
                    pf, featT[:, i * P:(i + 1) * P], s_w33, start=True, stop=False
                )
                nc.tensor.matmul(pf, s_ident, m_t, start=False, stop=False)
                nc.tensor.matmul(pf, s_ident, u1, start=False, stop=False)
                nc.tensor.matmul(pf, s_ident, u2, start=False, stop=True)

                # escape PSUM -> SBUF (split DVE / ACT), then DMA out
                o_t = opool.tile([P, DIM], F32)
                nc.vector.tensor_copy(o_t[:, :ESC], pf[:, :ESC])
                nc.scalar.copy(o_t[:, ESC:], pf[:, ESC:])
                nc.sync.dma_start(out=out[c * P:(c + 1) * P, :], in_=o_t)

    nc.compile()
    return nc


def host_prepare(msg, t, t_scale, t_shift, rope_log_ts, fourier_freqs,
                 fourier_W, fourier_b, log_decay, decay_bias, nt=NT,
                 n_cores=N_CORES):
    """Host-side constant prep + per-core sharding. Returns in_maps list."""
    e_pad = nt * P
    e_core = min(E_CORE, e_pad)
    a = float(np.asarray(t_scale).reshape(-1)[0]) / (math.sqrt(1.0) + 1e-6)
    b = float(np.asarray(t_shift).reshape(-1)[0])
    tn = (a * np.asarray(t, np.float64) + b).astype(np.float32)

    w = (1.0 / np.exp(np.asarray(rope_log_ts, np.float64))).astype(np.float32)
    w = w.reshape(-1)  # [256] hk order
    wrow = np.ascontiguousarray(np.broadcast_to(w, (P, NHK)))

    w33 = np.vstack([np.asarray(fourier_W, np.float32),
                     np.asarray(fourier_b, np.float32)[None, :]])
    fr = np.asarray(fourier_freqs, np.float64)
    fc = np.concatenate([fr, fr, [0.0]])[:, None]          # [33,1]
    fb = np.concatenate([np.zeros(NF), np.full(NF, HALF_PI),
                         [HALF_PI]])[:, None]              # [33,1]
    lam = np.exp(np.asarray(log_decay, np.float64)).astype(np.float32)
    pl2 = np.ascontiguousarray(np.broadcast_to(lam / 2.0, (P, H))).astype(np.float32)
    bh2 = np.ascontiguousarray(
        np.broadcast_to(np.asarray(decay_bias, np.float32) / 2.0, (P, H)))
    ident = np.eye(P, dtype=np.float32)

    consts = dict(wrow=wrow, w33=w33, ident=ident, pl2=pl2, bh2=bh2)

    msg = np.asarray(msg, np.float32)
    in_maps = []
    for ci in range(n_cores):
        lo = ci * e_core
        msh = msg[lo:lo + e_core]
        tsh = tn[lo:lo + e_core]
        if msh.shape[0] < e_pad:
            msh = np.concatenate(
                [msh, np.zeros((e_pad - msh.shape[0], DIM), np.float32)])
            tsh = np.concatenate([tsh, np.zeros(e_pad - tsh.shape[0], np.float32)])

        def cm(x):
            return np.ascontiguousarray(x.reshape(nt, P).T.astype(np.float32))

        ts64 = tsh.astype(np.float64)
        # cos reflection Sin(pi/2 - |tn|w) is only valid for |tn| <= 3pi/2;
        # clamp on-device args (keeps the spline in-domain) — edges beyond
        # the clamp are recomputed exactly on the host afterwards.
        tabs_clamped = np.minimum(np.abs(tsh), CLAMP)
        y = np.mod(ts64[None, :] * fc + fb + PI, TWO_PI).astype(np.float32)
        psign = (PI * np.sign(tsh)).astype(np.float32)
        in_maps.append(dict(
            msg=np.ascontiguousarray(msh),
            tn_cm=cm(tsh),
            tnneg_cm=cm(-tsh),
            tnabsneg_cm=cm(-tabs_clamped),
            pisign_cm=cm(psign),
            pisignneg_cm=cm(-psign),
            yrows=np.ascontiguousarray(y),
            **consts))
    return in_maps


def _exact_rows(msg_rows, tn_vals, rope_log_ts, fourier_freqs, fourier_W,
                fourier_b, log_decay, decay_bias):
    """Exact (float64) reference for a handful of rows; used to patch edges
    whose |tn| exceeds the on-device clamp."""
    w = 1.0 / np.exp(np.asarray(rope_log_ts, np.float64).reshape(-1))  # [256]
    tn = np.asarray(tn_vals, np.float64)
    ang = tn[:, None] * w[None, :]
    c, s = np.cos(ang), np.sin(ang)
    m = np.asarray(msg_rows, np.float64).reshape(-1, NHK, 2)
    me, mo = m[:, :, 0], m[:, :, 1]
    rot = np.stack([me * c - mo * s, me * s + mo * c], -1)
    phi = tn[:, None] * np.asarray(fourier_freqs, np.float64)[None, :]
    feat = np.concatenate([np.sin(phi), np.cos(phi)], -1)
    fourier = feat @ np.asarray(fourier_W, np.float64) + np.asarray(
        fourier_b, np.float64)
    lam = np.exp(np.asarray(log_decay, np.float64))
    g = 1.0 / (1.0 + np.exp(lam[None, :] * np.abs(tn)[:, None]
                            - np.asarray(decay_bias, np.float64)[None, :]))
    g2 = np.repeat(g, DIM // H, axis=1).reshape(-1, NHK, 2)
    outr = (g2 * rot + (1.0 - g2) * m).reshape(-1, DIM) + fourier
    return outr.astype(np.float32)


_NC = None


def kernel(**inputs) -> np.ndarray:
    global _NC
    if _NC is None:
        _NC = build_nc()
    from concourse.bass_utils import run_bass_kernel_spmd
    in_maps = host_prepare(**inputs)
    res = run_bass_kernel_spmd(_NC, in_maps, core_ids=list(range(N_CORES)))
    outs = [res.results[i]["out"][:E_CORE] for i in range(N_CORES)]
    out = np.concatenate(outs, axis=0)

    # host patch for the few edges beyond the on-device clamp
    a = float(np.asarray(inputs["t_scale"]).reshape(-1)[0]) / (1.0 + 1e-6)
    b = float(np.asarray(inputs["t_shift"]).reshape(-1)[0])
    tn = a * np.asarray(inputs["t"], np.float64) + b
    bad = np.where(np.abs(tn) > CLAMP)[0]
    if bad.size:
        out[bad] = _exact_rows(
            np.asarray(inputs["msg"])[bad], tn[bad], inputs["rope_log_ts"],
            inputs["fourier_freqs"], inputs["fourier_W"], inputs["fourier_b"],
            inputs["log_decay"], inputs["decay_bias"])
    return out
